# revision 55
# baseline (speedup 1.0000x reference)
"""Causal self-attention (GQA, QK-RMSNorm, partial RoPE, per-head gain) on 8 TRN2 cores.

Problem: B=4, T=2048, D=512; 8 q heads / 4 kv heads, head_dim 64, rope dims 16.

Sharding: core c handles batch b=c//2 and head-group g=c%2 (4 q heads + 2 kv
heads). Each core computes a partial projection y_part = ctx_g @ w_proj[:, g].T
over its 256 head dims; the host sums the two partials per batch.

On-device layout is fully "transposed" (feature-major) to keep every matmul
full-speed and avoid on-chip transposition of the attention probabilities:
  scores^T[kt, q] = k^T q   (kt on partitions, q on free axis)
  softmax without max-subtraction (s <= 40 << 88, exp cannot overflow fp32)
  ctx^T[d, q] accumulated as (v | ones)^T @ P  -- the 64 ones-columns make the
  matmul emit the softmax denominator replicated across 64 partitions for free.

Engine balance: ACT runs *only* the softmax exps (one wide exp per kt-block
covering both kv halves); the causal mask is applied pre-exp by accumulating a
precomputed -30k triangular matrix into the score PSUM with a tiny bf16
matmul (PE), so neither Pool nor ACT sits on the score->prob critical path.
RMS stats/normalize run on DVE (rsqrt via bit-trick + 2 Newton steps), RoPE
and PSUM->SBUF eviction copies on Pool. The AV matmul for block k is emitted
after the score matmul of block k+1 so PE never stalls on an exp; front
(QKV+norm+rope) and transpose work for future chunks is drip-fed between
attention blocks to fill the remaining PE slack.
"""

import numpy as np

import concourse.bass as bass
import concourse.mybir as mybir
import concourse.tile as tile
from concourse import bacc, bass_utils
from concourse.bass import ts
from concourse.masks import make_causal_mask, make_identity

P = 128
T = 2048
D = 512
NT = T // P          # 16 t-tiles
NQC = T // 512       # 4 query chunks of 512
HD = 64
ROPE_HALF = 8
EPS = float(np.finfo(np.float32).eps)
MASK_VAL = -30000.0

F32 = mybir.dt.float32
F32R = mybir.dt.float32r
BF16 = mybir.dt.bfloat16
I32 = mybir.dt.int32
AX = mybir.AxisListType
AF = mybir.ActivationFunctionType
ALU = mybir.AluOpType

_CACHE = {}


def _build(phases=(1, 2, 3)):
    nc = bacc.Bacc("TRN2", target_bir_lowering=False, debug=False)

    xT = nc.dram_tensor("xT", [D, T], F32, kind="ExternalInput").ap()
    wqkvT = nc.dram_tensor("wqkvT", [D, 512], F32, kind="ExternalInput").ap()
    wpT = nc.dram_tensor("wpT", [256, 512], F32, kind="ExternalInput").ap()
    cs = nc.dram_tensor("cs", [T, ROPE_HALF], F32, kind="ExternalInput").ap()
    sn = nc.dram_tensor("sn", [T, ROPE_HALF], F32, kind="ExternalInput").ap()
    gsc = nc.dram_tensor("gsc", [6], F32, kind="ExternalInput").ap()
    yT = nc.dram_tensor("yT", [D, T], F32, kind="ExternalOutput").ap()

    with tile.TileContext(nc) as tc:
        with tc.tile_pool(name="persist", bufs=1) as persist:
            # weights/tables first: every front matmul depends on wqkv.
            # dk-sliced so the first accumulation matmul starts after 256KB.
            wqkv_sb = persist.tile([P, 4, 512], F32R)
            wqkvv = wqkvT.bitcast(F32R).rearrange("(o p) m -> p o m", p=P)
            nc.sync.dma_start(wqkv_sb[:, 0], wqkvv[:, 0])
            xt_sb = persist.tile([P, 4, T], F32R)
            xTv = xT.bitcast(F32R).rearrange("(o p) t -> p o t", p=P)
            nc.sync.dma_start(xt_sb[:, :, ts(0, P)], xTv[:, :, ts(0, P)])
            for dk in range(1, 4):
                nc.sync.dma_start(wqkv_sb[:, dk], wqkvv[:, dk])
            for it in range(1, 4):
                nc.sync.dma_start(
                    xt_sb[:, :, ts(it, P)], xTv[:, :, ts(it, P)]
                )
            cs_sb = persist.tile([P, NT, ROPE_HALF], F32)
            nc.sync.dma_start(cs_sb[:], cs.rearrange("(n p) f -> p n f", p=P))
            sn_sb = persist.tile([P, NT, ROPE_HALF], F32)
            nc.sync.dma_start(sn_sb[:], sn.rearrange("(n p) f -> p n f", p=P))
            gsc_sb = persist.tile([P, 6], F32)
            nc.sync.dma_start(gsc_sb[:], gsc[None, :].to_broadcast((P, 6)))
            for it in range(4, NT):
                nc.sync.dma_start(
                    xt_sb[:, :, ts(it, P)], xTv[:, :, ts(it, P)]
                )
            wp_sb = persist.tile([P, 2, 512], F32R)
            nc.sync.dma_start(
                wp_sb[:], wpT.bitcast(F32R).rearrange("(o p) m -> p o m", p=P)
            )
            identb = persist.tile([P, P], BF16)
            make_identity(nc, identb[:])
            ident = persist.tile([P, P], F32)
            make_identity(nc, ident[:])
            # maskb[j, kt] = MASK_VAL where kt > j: accumulated into the
            # transposed-score PSUM of diagonal blocks (as maskb.T @ I) it
            # lands MASK_VAL on kt > q, so exp() zeroes the upper triangle.
            maskb = persist.tile([P, P], BF16)
            make_causal_mask(nc, maskb[:], MASK_VAL)
            magic_sb = persist.tile([P, 12], I32)
            nc.vector.memset(magic_sb[:], 0x5F3759DF)

            # qkT[:, 0] (qTa): head0 (kv0) rows 0:64, head2 (kv1) rows 64:128
            # qkT[:, 1] (qTb): head1 (kv0) rows 0:64, head3 (kv1) rows 64:128
            # qkT[:, 2] (kT):  kv0 rows 0:64, kv1 rows 64:128
            qkT = persist.tile([P, 3, T], F32R)
            # v_sb per t-tile columns: [v_kv0(64) | ones(64) | v_kv1(64) | ones(64)]
            # bf16: the AV matmul weight-loads v every block; bf16 enables the
            # PE fast-weight-load path (4x faster LDWEIGHTS on hardware)
            v_sb = persist.tile([P, NT, 256], BF16)
            v4 = v_sb.rearrange("p n (kv c) -> p n kv c", kv=2)
            nc.gpsimd.memset(v4[:, :, :, HD:128], 1.0)
            # ctxA: heads (4g+0, 4g+1); ctxB: heads (4g+2, 4g+3) -- matches wpT rows
            ctxA = persist.tile([P, T], F32R)
            ctxB = persist.tile([P, T], F32R)

            # PSUM budget (8 banks): fps 1 + tps 1 + sps 2x2 + ctx 2 = 8.
            # y_ps borrows the sps slots (proj runs while scores are idle).
            with (
                tc.tile_pool(name="fps", bufs=1, space="PSUM") as fps,
                tc.tile_pool(name="tps", bufs=1, space="PSUM") as tps,
                tc.tile_pool(name="sps", bufs=2, space="PSUM") as sps,
                tc.tile_pool(name="cps", bufs=1, space="PSUM") as cps,
                tc.tile_pool(name="qknp", bufs=4) as qknp,
                tc.tile_pool(name="smp", bufs=4) as smp,
                tc.tile_pool(name="pp", bufs=6) as pp,
                tc.tile_pool(name="ysb", bufs=4) as ysb,
            ):
                front_ps = {}
                front_qkn = {}

                def emit_qkv_mm(it, dk, pool=None):
                    """One of the 4 accumulating QKV matmuls for t-tile it.
                    The prologue alternates tiles between the fps and tps
                    slots (tr allocations only start after the prologue
                    fronts), so tile t+1's matmuls never wait on tile t's
                    drain while there is no attention work to overlap."""
                    if dk == 0:
                        front_ps[it] = (pool or fps).tile(
                            [P, 512], F32, tag="qkv" if pool is None else "tr",
                            name="qkv_ps"
                        )
                    nc.tensor.matmul(
                        front_ps[it][:],
                        xt_sb[:, dk, ts(it, P)],
                        wqkv_sb[:, dk, :],
                        start=(dk == 0),
                        stop=(dk == 3),
                    )

                def emit_drain(it):
                    """Evict t-tile it's QKV PSUM: q/k rows into its half of
                    the pair's qkn2 tile, v into v_sb. Early tiles drain via
                    ACT: the run-in is regionally DVE-bound while ACT still
                    has exp slack there."""
                    qkv_ps = front_ps.pop(it)
                    if it % 2 == 0:
                        front_qkn[it // 2] = qknp.tile(
                            [P, 2, 384], F32, tag="qkn", name="qkn2"
                        )
                    qkn2 = front_qkn[it // 2]
                    nc.scalar.copy(qkn2[:, it % 2, :], qkv_ps[:, 0:384])
                    # v (no norm): straight into v_sb with the ones-gap
                    # layout; always DVE so the two drain copies overlap
                    nc.vector.tensor_copy(
                        v4[:, it, :, 0:HD],
                        qkv_ps[:, 384:512].rearrange("p (kv c) -> p kv c", c=HD),
                    )

                def emit_front_stats(u):
                    """RMS-norm + RoPE for the tile pair (2u, 2u+1), batched
                    double-wide on DVE (square on Pool). Touches no PE/ACT."""
                    qkn2 = front_qkn[u]
                    sq = smp.tile([P, 2, 384], F32, tag="sq", name="sq", bufs=2)
                    nc.gpsimd.tensor_mul(sq[:], qkn2[:], qkn2[:])
                    ms = smp.tile([P, 2, 6], F32, tag="ms", name="ms")
                    nc.vector.reduce_sum(
                        ms[:], sq.rearrange("p u (s c) -> p u s c", c=HD),
                        axis=AX.X,
                    )
                    # rstd = rsqrt(ms/64 + eps): int bit-trick + 2 Newton steps
                    nc.vector.tensor_scalar(
                        ms[:], ms[:], 1.0 / HD, EPS, op0=ALU.mult, op1=ALU.add
                    )
                    y = smp.tile([P, 2, 6], F32, tag="y", name="y")
                    nc.vector.tensor_scalar(
                        y.bitcast(I32)[:], ms.bitcast(I32)[:], 1, None,
                        op0=ALU.arith_shift_right,
                    )
                    nc.vector.tensor_tensor(
                        y.bitcast(I32)[:],
                        magic_sb.rearrange("p (u s) -> p u s", u=2)[:],
                        y.bitcast(I32)[:],
                        ALU.subtract,
                    )
                    nt_ = smp.tile([P, 2, 6], F32, tag="nt", name="nt_")
                    for _ in range(2):
                        nc.vector.tensor_mul(nt_[:], y[:], y[:])
                        nc.vector.tensor_mul(nt_[:], nt_[:], ms[:])
                        nc.vector.tensor_scalar(
                            nt_[:], nt_[:], -0.5, 1.5, op0=ALU.mult, op1=ALU.add
                        )
                        nc.vector.tensor_mul(y[:], y[:], nt_[:])
                    # fold per-head gain * 1/sqrt(head_dim) into q strips
                    nc.vector.tensor_tensor(
                        y[:], y[:],
                        gsc_sb[:, None, :].to_broadcast((P, 2, 6)),
                        ALU.mult,
                    )
                    qk4 = qkn2.rearrange("p u (s c) -> p u s c", c=HD)
                    nc.vector.tensor_tensor(
                        qk4[:],
                        qk4[:],
                        y[:, :, :, None].to_broadcast((P, 2, 6, HD)),
                        ALU.mult,
                    )
                    # RoPE on first 16 dims of each strip (DVE, double-wide)
                    x1 = qk4[:, :, :, 0:ROPE_HALF]
                    x2 = qk4[:, :, :, ROPE_HALF : 2 * ROPE_HALF]
                    cb = cs_sb[:, 2 * u : 2 * u + 2, None, :].to_broadcast(
                        (P, 2, 6, ROPE_HALF)
                    )
                    snb = sn_sb[:, 2 * u : 2 * u + 2, None, :].to_broadcast(
                        (P, 2, 6, ROPE_HALF)
                    )
                    t1 = smp.tile([P, 2, 6, ROPE_HALF], F32, tag="t1", name="t1")
                    t2 = smp.tile([P, 2, 6, ROPE_HALF], F32, tag="t2", name="t2")
                    nc.vector.tensor_mul(t1[:], x1, snb)
                    nc.vector.tensor_mul(x1, x1, cb)
                    nc.vector.tensor_mul(t2[:], x2, snb)
                    nc.vector.tensor_tensor(x1, x1, t2[:], ALU.add)
                    nc.vector.tensor_mul(x2, x2, cb)
                    nc.vector.tensor_tensor(x2, x2, t1[:], ALU.subtract)

                def emit_transpose(it):
                    """Transpose all 3 128-col strips of tile it's normalized
                    qk into one PSUM bank (start/stop chained), single evict."""
                    qkn = front_qkn[it // 2][:, it % 2, :]
                    tr = tps.tile([P, 3, P], F32, tag="tr", name="tr")
                    for j in range(3):
                        nc.tensor.matmul(
                            tr[:, j, :],
                            qkn[:, j * P : (j + 1) * P],
                            ident[:],
                            is_transpose=True,
                            start=(j == 0),
                            stop=(j == 2),
                        )
                    nc.vector.tensor_copy(qkT[:, :, ts(it, P)], tr[:])
                    if it % 2 == 1:
                        del front_qkn[it // 2]

                # ---- filler queue: front + transpose units for tiles 4..15,
                # drip-fed into the attention inner loop so PE slack between
                # exp-gated AV matmuls does useful work. Transposes of tile
                # it-1 interleave with QKV matmuls of tile it so the tps
                # single-buffer round-trip (PE -> Pool -> PE) never stalls PE.
                fillers = []
                c_pos = {}

                def _add_c(t):
                    fillers.append(lambda t=t: emit_transpose(t))
                    c_pos[t] = len(fillers)

                for u in range(2, NT // 2):
                    for t in (2 * u, 2 * u + 1):
                        for dk in range(4):
                            fillers.append(
                                lambda t=t, dk=dk: emit_qkv_mm(t, dk)
                            )
                        fillers.append(lambda t=t: emit_drain(t))
                        if u > 2:
                            # transposes ride one pair behind their front so
                            # the PE never waits on an unfinished DVE chain
                            _add_c(t - 2)
                    fillers.append(lambda u=u: emit_front_stats(u))
                _add_c(NT - 2)
                _add_c(NT - 1)

                # deadline[qc] = fillers that must be emitted before
                # attention(qc) starts (tiles 4qc..4qc+3 transposed)
                def _deadline(qc):
                    return max(
                        (c_pos[t] for t in range(4 * (qc + 1)) if t in c_pos),
                        default=0,
                    )
                fill_pos = [0]

                def fill(target):
                    while fill_pos[0] < min(target, len(fillers)):
                        fillers[fill_pos[0]]()
                        fill_pos[0] += 1

                def emit_av(ktb, qlo, pb, ctxs, nkt):
                    p3 = pb.rearrange("p (u c) -> p u c", u=2)
                    for s_u in (0, 1):
                        nc.tensor.matmul(
                            ctxs[s_u][:, qlo:512],
                            v_sb[:, ktb, 128 * s_u : 128 * s_u + 128],
                            p3[:, s_u, qlo:512],
                            start=(ktb == 0),
                            stop=(ktb == nkt - 1),
                        )

                def emit_attention_pair(qc, s_t, fill_from, fill_until,
                                        head_work=()):
                    """Both kv-halves of one qT tile. Per kt-block: 2 score
                    matmuls (one per kv half) into one 2-bank PSUM tile, a
                    bf16 mask matmul-accumulate on diagonal blocks, one wide
                    exp, and the (pipelined, one block delayed) AV pair."""
                    nkt = 4 * (qc + 1)
                    ctxs = [
                        cps.tile([P, 512], F32, tag=f"ctx{s_u}",
                                 name=f"ctx{s_u}")
                        for s_u in (0, 1)
                    ]
                    # AV matmuls trail the score/exp stream by two blocks:
                    # exp(k) + two sem hops take ~1.3us, more than one block
                    # of PE work, so a one-block delay still stalls PE
                    av_q = []
                    for ktb in range(nkt):
                        d = ktb - 4 * qc
                        qlo = max(0, d) * P
                        # f32r matmuls below 256 columns run at 1/4 rate:
                        # never issue a score matmul narrower than 256.
                        slo = min(qlo, 256)
                        s_ps = sps.tile([P, 1024], F32, tag="s_ps", name="s_ps")
                        s3 = s_ps.rearrange("p (u c) -> p u c", u=2)
                        for s_u in (0, 1):
                            kb = HD * s_u
                            nc.tensor.matmul(
                                s3[:, s_u, slo:512],
                                qkT[kb : kb + HD, 2, ts(ktb, P)],
                                qkT[kb : kb + HD, s_t, qc * 512 + slo : qc * 512 + 512],
                                start=True,
                                stop=(d < 0),
                            )
                        if d >= 0:
                            for s_u in (0, 1):
                                nc.tensor.matmul(
                                    s3[:, s_u, qlo : qlo + P],
                                    maskb[:],
                                    identb[:],
                                    start=False,
                                    stop=True,
                                )
                        pb = pp.tile([P, 1024], BF16, tag="pb", name="pb")
                        p3 = pb.rearrange("p (u c) -> p u c", u=2)
                        nc.scalar.activation(
                            p3[:, :, qlo:512], s3[:, :, qlo:512], AF.Exp
                        )
                        av_q.append((ktb, qlo, pb, ctxs, nkt))
                        if len(av_q) > 2:
                            emit_av(*av_q.pop(0))
                        if ktb % 2 == 1 and ktb // 2 < len(head_work):
                            # previous chunk's projection, one output tile
                            # every second block: y_ps borrows an sps slot, so
                            # consecutive score blocks must stay adjacent in
                            # the rotation or exp latency stalls the scores
                            head_work[ktb // 2]()
                        # pace fillers to finish two blocks early (their
                        # trailing transposes depend on fresh DVE chains), but
                        # hold ~25% back mid-queue to cover the boundary
                        frac = min(1.0, (ktb + 1) / max(1, nkt - 2))
                        if fill_until < len(fillers):
                            frac *= 0.75
                        fill(fill_from + int(frac * (fill_until - fill_from)))
                    for item in av_q:
                        emit_av(*item)
                    for s_u in (0, 1):
                        dst = ctxA if s_u == 0 else ctxB
                        rb = HD * s_t
                        linv = smp.tile([HD, 512], F32, tag="linv", name="linv",
                                        bufs=2)
                        nc.vector.reciprocal(linv[:], ctxs[s_u][HD:P, :])
                        nc.vector.tensor_tensor(
                            dst[rb : rb + HD, ts(qc, 512)],
                            ctxs[s_u][0:HD, :],
                            linv[:],
                            ALU.mult,
                        )

                def emit_proj_ot(qc, ot, split=False):
                    # borrow an sps slot: scores rotate around it block-wise
                    y_ps = sps.tile([P, 512], F32, tag="s_ps", name="y_ps")
                    for mt, src in ((0, ctxA), (1, ctxB)):
                        nc.tensor.matmul(
                            y_ps[:],
                            wp_sb[:, mt, ts(ot, P)],
                            src[:, ts(qc, 512)],
                            start=(mt == 0),
                            stop=(mt == 1),
                        )
                    y_sb = ysb.tile([P, 512], F32, tag="y_sb", name="y_sb")
                    yrow = yT[ot * P : (ot + 1) * P, qc * 512 : (qc + 1) * 512]
                    if split:
                        nc.scalar.copy(y_sb[:], y_ps[:])
                    else:
                        nc.vector.tensor_copy(y_sb[:], y_ps[:])
                    nc.sync.dma_start(yrow, y_sb[:])

                do1 = 1 in phases
                do2 = 2 in phases
                do3 = 3 in phases
                if do1:
                    for u in (0, 1):
                        for t in (2 * u, 2 * u + 1):
                            for dk in range(4):
                                emit_qkv_mm(t, dk, pool=tps if t % 2 else None)
                            emit_drain(t)
                        emit_front_stats(u)
                    for it in range(4):
                        emit_transpose(it)

                if do2:
                    pending_proj = []
                    for qc in range(NQC):
                        lo, hi = _deadline(qc), _deadline(qc + 1)
                        for s_t in (0, 1):
                            a = lo + (hi - lo) * (1 + s_t) // 2
                            emit_attention_pair(
                                qc, s_t,
                                lo if s_t == 0 else a,
                                a if s_t == 0 else hi,
                                head_work=pending_proj if s_t == 0 else (),
                            )
                            if s_t == 0:
                                pending_proj = []
                        fill(hi)
                        if do3:
                            pending_proj = [
                                (lambda qc=qc, ot=ot: emit_proj_ot(qc, ot))
                                for ot in range(4)
                            ]
                    # last chunk's projection has no following attention to
                    # hide in: flush it, alternating eviction engines
                    for ot in range(4):
                        emit_proj_ot(NQC - 1, ot, split=(ot % 2 == 1))

                    pending_proj = []
                if do1:
                    fill(len(fillers))

    nc.compile()
    return nc


def _host_inputs(x, w_q, w_k, w_v, w_proj, q_gain):
    """Build the 8 per-core input maps."""
    B = x.shape[0]
    inv_freq = 1.0 / (
        10000.0 ** (np.arange(0, 16, 2, dtype=np.float32) / np.float32(16.0))
    )
    freqs = np.outer(np.arange(T, dtype=np.float32), inv_freq)
    cs = np.cos(freqs).astype(np.float32)
    sn = np.sin(freqs).astype(np.float32)

    per_group = []
    for g in range(2):
        heads = [4 * g + 0, 4 * g + 2, 4 * g + 1, 4 * g + 3]
        qrows = np.concatenate([w_q[64 * h : 64 * h + 64] for h in heads], axis=0)
        krows = w_k[128 * g : 128 * g + 128]
        vrows = w_v[128 * g : 128 * g + 128]
        wqkvT = np.ascontiguousarray(
            np.concatenate([qrows, krows, vrows], axis=0).T.astype(np.float32)
        )
        wpT = np.ascontiguousarray(
            w_proj[:, 256 * g : 256 * g + 256].T.astype(np.float32)
        )
        gains = np.array(
            [q_gain[h] * 0.125 for h in heads] + [1.0, 1.0], dtype=np.float32
        )
        per_group.append((wqkvT, wpT, gains))

    in_maps = []
    for c in range(8):
        b, g = c // 2, c % 2
        wqkvT, wpT, gains = per_group[g]
        in_maps.append(
            {
                "xT": np.ascontiguousarray(x[b].T.astype(np.float32)),
                "wqkvT": wqkvT,
                "wpT": wpT,
                "cs": cs,
                "sn": sn,
                "gsc": gains,
            }
        )
    return in_maps


def kernel(x, w_q, w_k, w_v, w_proj, q_gain, _trace=False):
    x = np.asarray(x, dtype=np.float32)
    w_q = np.asarray(w_q, dtype=np.float32)
    w_k = np.asarray(w_k, dtype=np.float32)
    w_v = np.asarray(w_v, dtype=np.float32)
    w_proj = np.asarray(w_proj, dtype=np.float32)
    q_gain = np.asarray(q_gain, dtype=np.float32)

    if "nc" not in _CACHE:
        _CACHE["nc"] = _build()
    nc = _CACHE["nc"]

    in_maps = _host_inputs(x, w_q, w_k, w_v, w_proj, q_gain)
    res = bass_utils.run_bass_kernel_spmd(
        nc, in_maps, core_ids=list(range(8)), trace=_trace
    )
    _CACHE["last_result"] = res

    B = x.shape[0]
    y = np.empty((B, T, D), dtype=np.float32)
    for b in range(B):
        yT = res.results[2 * b]["yT"] + res.results[2 * b + 1]["yT"]
        y[b] = yT.T
    return y
